# revision 19
# baseline (speedup 1.0000x reference)
"""Trainium2 Bass kernel for nn_ConvolutionalLayer_83751862272232.

5x5 SAME conv (NHWC 1x512x512x64 -> 1x512x512x128) + membrane potential
accumulation + spike threshold/reset + lateral inhibition.

Distribution: H sharded across 8 cores (64 rows each), halos resolved on the
host by giving each core a 68-row padded channel-major input slice.  No
cross-core communication.

Per core, conv is computed one output row at a time into a PSUM bank
[128co x 512w] as a sum of per-tap matmuls (contraction = input channels).
Taps are processed two-at-a-time by stacking two input rows' channels on the
128-partition contraction dim (15 matmuls of K=128 instead of 25 of K=64,
with zero-padded weight variants for odd/even rows).

Precision: fp32 matmuls on the PE run at 4 cycles/row; float32r runs at
1 cycle/row but truncates operands to ~11 mantissa bits, which flips spikes
near the 15.0 threshold.  Default mode "f32r_cross" feeds the main pass
host-pre-truncated (m11) operands — so the PE's internal truncation is a
no-op — and adds a bf16 cross-term pass (xres*wr + xr*wres, 25 matmuls of
K=128) accumulated into the same PSUM bank.  Measured on CPU: ~7 spike flips
out of 33.5M vs jax fp32 reference.

Post-conv per row: spikes = (pot >= 15) [* (k_inh>0)]; pot_out = pot*(pot<15);
column spike-count via a ones-vector matmul -> k_inh_new row.  (The
reference's lateral-inhibition max comparison is vacuous because the reset
writes 0 to every spiking channel before the max is taken, so all
non-inhibited spikes survive.)
"""

import os
import sys
from contextlib import ExitStack

import numpy as np

for _p in ("/opt/trn_rl_repo", "/root/.axon_site/_ro/trn_rl_repo"):
    if os.path.isdir(_p) and _p not in sys.path:
        sys.path.append(_p)

import ml_dtypes  # noqa: E402

import concourse.bass as bass  # noqa: E402
import concourse.tile as tile  # noqa: E402
from concourse import bacc, bass_utils, mybir  # noqa: E402

H = W = 512
CI = 64
CO = 128
NCORES = 8
HS = H // NCORES          # 64 output rows per core
KH = KW = 5
PAD = 2
WP = W + 2 * PAD          # 516 padded width
HROWS = HS + KH - 1       # 68 padded input rows per core
NPAIRS = (HROWS + 1) // 2  # 34 row-pair tiles
THRESHOLD = 15.0

F32 = mybir.dt.float32
F32R = mybir.dt.float32r
BF16 = mybir.dt.bfloat16

# "f32r_cross" | "f32r" | "f32"
MODE = os.environ.get("KERNEL_MODE", "f32r_cross")

bf16_np = ml_dtypes.bfloat16


def _trunc_m11(a: np.ndarray) -> np.ndarray:
    """Truncate fp32 mantissa to 11 bits (the PE's fp32r operand precision)."""
    u = np.ascontiguousarray(a, dtype=np.float32).view(np.uint32)
    return (u & np.uint32(0xFFFFF000)).view(np.float32)


def build_program(use_oldpot: bool, use_kinh: bool, mode: str):
    nc = bacc.Bacc("TRN2", target_bir_lowering=False, debug=False,
                   num_devices=NCORES)

    mm_in_dt = F32 if mode == "f32" else F32R
    xp_d = nc.dram_tensor("xp", [NPAIRS, 2 * CI, WP], mm_in_dt,
                          kind="ExternalInput").ap()
    wm_d = nc.dram_tensor("wm", [2 * CI, 2, 3, KW, CO], mm_in_dt,
                          kind="ExternalInput").ap()
    ones_d = nc.dram_tensor("ones", [2 * CI, 1], mm_in_dt,
                            kind="ExternalInput").ap()
    cross = mode == "f32r_cross"
    if cross:
        xc_d = nc.dram_tensor("xc", [HROWS, 2 * CI, WP], BF16,
                              kind="ExternalInput").ap()
        wc_d = nc.dram_tensor("wc", [2 * CI, KH * KW, CO], BF16,
                              kind="ExternalInput").ap()
    if use_oldpot:
        op_d = nc.dram_tensor("oldpot", [CO, HS, W], F32,
                              kind="ExternalInput").ap()
    if use_kinh:
        ki_d = nc.dram_tensor("kinh_in", [HS, W], F32,
                              kind="ExternalInput").ap()
    pot_d = nc.dram_tensor("pot", [CO, HS, W], F32, kind="ExternalOutput").ap()
    spk_d = nc.dram_tensor("spk", [CO, HS, W], mm_in_dt,
                           kind="ExternalOutput").ap()
    kout_d = nc.dram_tensor("kout", [HS, W], F32, kind="ExternalOutput").ap()

    # fp32r operands must be PRODUCED as float32r (DMA/DVE rounds to FP22 at
    # the producer; the BIR verifier enforces it), so tiles holding fp32r
    # matmul operands are declared float32r, not bitcast.
    mm_dt = F32 if mode == "f32" else F32R
    is_ge = mybir.AluOpType.is_ge
    is_lt = mybir.AluOpType.is_lt
    mult = mybir.AluOpType.mult
    add = mybir.AluOpType.add

    with tile.TileContext(nc) as tc, ExitStack() as ctx:
        xpool = ctx.enter_context(tc.tile_pool(name="x", bufs=NPAIRS))
        wpool = ctx.enter_context(tc.tile_pool(name="w", bufs=1))
        outp = ctx.enter_context(tc.tile_pool(name="outs", bufs=3))
        krp = ctx.enter_context(tc.tile_pool(name="kr", bufs=4))
        accp = ctx.enter_context(tc.tile_pool(name="acc", bufs=4, space="PSUM"))
        cntp = ctx.enter_context(tc.tile_pool(name="cnt", bufs=4, space="PSUM"))
        if cross:
            xcpool = ctx.enter_context(tc.tile_pool(name="xc", bufs=HROWS))

        w_sb = wpool.tile([2 * CI, 2, 3, KW, CO], mm_in_dt)
        nc.sync.dma_start(out=w_sb[:], in_=wm_d[:])
        ones_sb = wpool.tile([2 * CI, 1], mm_in_dt)
        nc.sync.dma_start(out=ones_sb[:], in_=ones_d[:])
        if cross:
            wc_sb = wpool.tile([2 * CI, KH * KW, CO], BF16)
            nc.sync.dma_start(out=wc_sb[:], in_=wc_d[:])

        xtiles = []
        for p in range(NPAIRS):
            xt = xpool.tile([2 * CI, WP], mm_in_dt)
            nc.sync.dma_start(out=xt[:], in_=xp_d[p])
            xtiles.append(xt)
        xctiles = []
        if cross:
            for r in range(HROWS):
                xct = xcpool.tile([2 * CI, WP], BF16)
                nc.sync.dma_start(out=xct[:], in_=xc_d[r])
                xctiles.append(xct)

        for y in range(HS):
            p0, e = y // 2, y % 2
            acc = accp.tile([CO, W], F32)
            n_mm = 15 + (int(os.environ.get("CROSS_N", "25")) if cross else 0)
            mm_i = 0
            for k in range(3):
                rhs_t = xtiles[p0 + k]
                for dx in range(KW):
                    nc.tensor.matmul(
                        acc[:],
                        w_sb[:, e, k, dx, :],
                        rhs_t[:, dx:dx + W],
                        start=(mm_i == 0), stop=(mm_i == n_mm - 1))
                    mm_i += 1
            if cross:
                cross_n = int(os.environ.get("CROSS_N", "25"))
                for ky in range(KH):
                    rhs_c = xctiles[y + ky]
                    for dx in range(KW):
                        if mm_i - 15 >= cross_n:
                            continue
                        nc.tensor.matmul(
                            acc[:],
                            wc_sb[:, ky * KW + dx, :],
                            rhs_c[:, dx:dx + W],
                            start=False, stop=(mm_i == n_mm - 1))
                        mm_i += 1

            if use_oldpot:
                old_sb = outp.tile([CO, W], F32)
                nc.sync.dma_start(out=old_sb[:], in_=op_d[:, y, :])
                pre_sb = outp.tile([CO, W], F32)
                nc.vector.tensor_tensor(out=pre_sb[:], in0=acc[:],
                                        in1=old_sb[:], op=add)
                src = pre_sb
            else:
                src = acc

            spk_sb = outp.tile([CO, W], mm_in_dt)
            nc.vector.tensor_scalar(out=spk_sb[:], in0=src[:],
                                    scalar1=THRESHOLD, scalar2=None, op0=is_ge)

            # pot = (not spiked) * pot; spk_sb is in SBUF so only one PSUM
            # operand (src) is read (HW limit: one PSUM input per DVE op).
            pot_sb = outp.tile([CO, W], F32)
            nc.vector.scalar_tensor_tensor(
                out=pot_sb[:], in0=spk_sb[:], scalar=0.5, op0=is_lt,
                in1=src[:], op1=mult)

            if use_kinh:
                # spikes survive only where k_inh > 0 (after pot reset above)
                ni_sb = outp.tile([CO, W], F32)
                bcast = bass.AP(tensor=ki_d.tensor, offset=ki_d.offset + y * W,
                                ap=[[0, CO], [1, W]])
                nc.sync.dma_start(out=ni_sb[:], in_=bcast)
                nc.vector.scalar_tensor_tensor(
                    out=spk_sb[:], in0=ni_sb[:], scalar=0.0,
                    op0=mybir.AluOpType.is_gt, in1=spk_sb[:], op1=mult)

            cnt = cntp.tile([1, W], F32)
            nc.tensor.matmul(cnt[:], ones_sb[:], spk_sb[:],
                             start=True, stop=True)
            kr = krp.tile([1, W], F32)
            if use_kinh:
                kin_sb = krp.tile([1, W], F32)
                nc.sync.dma_start(out=kin_sb[:], in_=ki_d[y:y + 1, :])
                nc.vector.scalar_tensor_tensor(
                    out=kr[:], in0=cnt[:], scalar=0.5, op0=is_lt,
                    in1=kin_sb[:], op1=mult)
            else:
                nc.vector.tensor_scalar(out=kr[:], in0=cnt[:], scalar1=0.5,
                                        scalar2=None, op0=is_lt)

            nc.sync.dma_start(out=pot_d[:, y, :], in_=pot_sb[:])
            nc.sync.dma_start(out=spk_d[:, y, :], in_=spk_sb[:])
            nc.sync.dma_start(out=kout_d[y:y + 1, :], in_=kr[:])

    nc.compile()
    return nc


_CACHE: dict = {}


def _get_program(use_oldpot: bool, use_kinh: bool, mode: str):
    key = (use_oldpot, use_kinh, mode)
    if key not in _CACHE:
        _CACHE[key] = build_program(use_oldpot, use_kinh, mode)
    return _CACHE[key]


def _prep_inputs(x, weights, old_potentials, k_inh, use_oldpot, use_kinh,
                 mode):
    """Host-side shard/pad/transpose. Returns per-core input maps."""
    cross = mode == "f32r_cross"
    x = np.ascontiguousarray(x, dtype=np.float32)
    weights = np.ascontiguousarray(weights, dtype=np.float32)

    if mode == "f32r_cross":
        xr = _trunc_m11(x)
        wr = _trunc_m11(weights)
    else:
        xr, wr = x, weights

    # x: [1,H,W,CI] -> channel-major padded [CI, H+4, W+4]
    xt = np.zeros((CI, H + 2 * PAD, WP), dtype=np.float32)
    xt[:, PAD:PAD + H, PAD:PAD + W] = xr[0].transpose(2, 0, 1)
    if cross:
        xres = (x - xr)[0].transpose(2, 0, 1)  # [CI, H, W]
        xct = np.zeros((2 * CI, H + 2 * PAD, WP), dtype=bf16_np)
        xct[:CI, PAD:PAD + H, PAD:PAD + W] = xres.astype(bf16_np)
        xct[CI:, PAD:PAD + H, PAD:PAD + W] = \
            xr[0].transpose(2, 0, 1).astype(bf16_np)

    # main weights: [2(parity), 3(pair), KW, 128(part: 2x64ci), CO]
    wstk = np.zeros((2, 3, KW, 2 * CI, CO), dtype=np.float32)
    for e in (0, 1):
        for k in range(3):
            ky_a = 2 * k - e
            for half, ky in ((0, ky_a), (1, ky_a + 1)):
                if 0 <= ky < KH:
                    # wr[ky] : [KW, CI, CO]
                    wstk[e, k, :, half * CI:(half + 1) * CI, :] = wr[ky]
    wm_host = np.ascontiguousarray(
        wstk.transpose(3, 0, 1, 2, 4))  # [128, 2, 3, KW, CO]

    if cross:
        wcs = np.zeros((KH * KW, 2 * CI, CO), dtype=bf16_np)
        wres = (weights - wr).astype(bf16_np)
        wr16 = wr.astype(bf16_np)
        for ky in range(KH):
            for dx in range(KW):
                wcs[ky * KW + dx, :CI, :] = wr16[ky, dx]
                wcs[ky * KW + dx, CI:, :] = wres[ky, dx]
        wc_host = np.ascontiguousarray(wcs.transpose(1, 0, 2))

    in_maps = []
    for c in range(NCORES):
        s = xt[:, c * HS:c * HS + HROWS, :]              # [CI, 68, WP]
        xp_core = np.ascontiguousarray(
            s.transpose(1, 0, 2).reshape(NPAIRS, 2 * CI, WP))
        m = {"xp": xp_core, "wm": wm_host,
             "ones": np.ones((2 * CI, 1), np.float32)}
        if cross:
            sc = xct[:, c * HS:c * HS + HROWS, :]        # [128, 68, WP]
            m["xc"] = np.ascontiguousarray(sc.transpose(1, 0, 2))
            m["wc"] = wc_host
        if use_oldpot:
            m["oldpot"] = np.ascontiguousarray(
                old_potentials[0, c * HS:(c + 1) * HS].transpose(2, 0, 1))
        if use_kinh:
            m["kinh_in"] = np.ascontiguousarray(
                k_inh[c * HS:(c + 1) * HS], dtype=np.float32)
        in_maps.append(m)
    return in_maps


LAST_RESULTS = None
LAST_EXEC_NS = None


def _run_pjrt(nc, in_maps, time_iters=0):
    """Execute the Bass program on the 8 axon-tunneled cores via PJRT.

    Modeled on bass2jax.run_bass_via_pjrt, but without output-buffer
    donation so the uploaded device arrays stay valid and repeated calls
    can be timed (our kernel writes every output element, so zero-init
    donation is unnecessary).
    """
    import jax
    from jax.sharding import Mesh, PartitionSpec
    from jax.experimental.shard_map import shard_map
    from concourse import bass2jax as b2j

    b2j.install_neuronx_cc_hook()

    partition_name = (nc.partition_id_tensor.name
                      if nc.partition_id_tensor else None)
    in_names, out_names, out_avals = [], [], []
    for alloc in nc.m.functions[0].allocations:
        if not isinstance(alloc, mybir.MemoryLocationSet):
            continue
        name = alloc.memorylocations[0].name
        if alloc.kind == "ExternalInput":
            if name != partition_name:
                in_names.append(name)
        elif alloc.kind == "ExternalOutput":
            out_names.append(name)
            out_avals.append(jax.core.ShapedArray(
                tuple(alloc.tensor_shape), mybir.dt.np(alloc.dtype)))
    n_params = len(in_names)
    all_names = in_names + out_names
    if partition_name is not None:
        all_names = all_names + [partition_name]

    def _body(*args):
        operands = list(args)
        if partition_name is not None:
            operands.append(b2j.partition_id_tensor())
        return tuple(b2j._bass_exec_p.bind(
            *operands,
            out_avals=tuple(out_avals),
            in_names=tuple(all_names),
            out_names=tuple(out_names),
            lowering_input_output_aliases=(),
            sim_require_finite=True,
            sim_require_nnan=True,
            nc=nc,
        ))

    devices = jax.devices()[:NCORES]
    mesh = Mesh(np.asarray(devices), ("core",))
    n_outs = len(out_names)
    sharded = jax.jit(
        shard_map(_body, mesh=mesh,
                  in_specs=(PartitionSpec("core"),) * (n_params + n_outs),
                  out_specs=(PartitionSpec("core"),) * n_outs,
                  check_rep=False),
        keep_unused=True)

    sharding = jax.sharding.NamedSharding(mesh, PartitionSpec("core"))
    dev_in = []
    for i, name in enumerate(in_names):
        cat = np.concatenate([np.asarray(m[name]) for m in in_maps], axis=0)
        dev_in.append(jax.device_put(cat, sharding))
    for av in out_avals:
        z = np.zeros((NCORES * av.shape[0], *av.shape[1:]), av.dtype)
        dev_in.append(jax.device_put(z, sharding))

    outs = sharded(*dev_in)
    jax.block_until_ready(outs)

    exec_ns = None
    if time_iters:
        import time
        # warmup already done; time K back-to-back executes
        t0 = time.perf_counter()
        for _ in range(time_iters):
            outs2 = sharded(*dev_in)
        jax.block_until_ready(outs2)
        t1 = time.perf_counter()
        exec_ns = (t1 - t0) / time_iters * 1e9

    results = [
        {name: np.asarray(outs[i]).reshape(NCORES, *out_avals[i].shape)[c]
         for i, name in enumerate(out_names)}
        for c in range(NCORES)
    ]
    return results, exec_ns


def run(x, weights, old_potentials, k_inh, trace=False, mode=None,
        time_iters=0):
    global LAST_RESULTS, LAST_EXEC_NS
    mode = mode or MODE
    x = np.asarray(x)
    weights = np.asarray(weights)
    old_potentials = np.asarray(old_potentials)
    k_inh = np.asarray(k_inh)

    use_oldpot = bool(np.any(old_potentials))
    use_kinh = bool(np.any(k_inh <= 0.0))

    nc = _get_program(use_oldpot, use_kinh, mode)
    in_maps = _prep_inputs(x, weights, old_potentials, k_inh,
                           use_oldpot, use_kinh, mode)
    results, exec_ns = _run_pjrt(nc, in_maps, time_iters=time_iters)
    LAST_RESULTS = results
    LAST_EXEC_NS = exec_ns

    pot = np.concatenate([results[c]["pot"] for c in range(NCORES)],
                         axis=1)               # [CO, H, W]
    spk = np.concatenate([results[c]["spk"] for c in range(NCORES)],
                         axis=1)
    kout = np.concatenate([results[c]["kout"] for c in range(NCORES)],
                          axis=0)              # [H, W]

    spikes = np.ascontiguousarray(
        spk.transpose(1, 2, 0))[None].astype(np.float32)
    pot_out = np.ascontiguousarray(
        pot.transpose(1, 2, 0))[None].astype(np.float32)
    return spikes, pot_out, kout.astype(np.float32)


def kernel(x, weights, old_potentials, k_inh):
    return run(x, weights, old_potentials, k_inh,
               trace=bool(os.environ.get("BASS_TRACE")))
